# revision 2
# baseline (speedup 1.0000x reference)
"""D4-pool Trainium2 kernel.

x: [256, 128, 64, 64] f32. Groups of 8 consecutive batch entries hold the 8
D4 orientations of one image; undo each orientation and mean over the group,
giving [32, 128, 64, 64].

Sharding: data-parallel over the group dim — core k gets groups [4k, 4k+4)
(batch entries [32k, 32k+32)), so the reduce is fully device-local.

Layout trick: with C (=128) on SBUF partitions and (H, W) on the free dim,
every D4 inverse transform is pure free-dim address arithmetic (stride ±1 /
±64 access patterns). Per partition, the required inverse-transform reads:
  o=0: A[h, w]          o=1: A[w, 63-h]     o=2: A[63-h, 63-w]
  o=3: A[63-w, h]       o=4: A[h, 63-w]     o=5: A[w, h]
  o=6: A[63-h, w]       o=7: A[63-w, 63-h]

Two accumulators so only ONE DVE op per group pays the slow inner-stride-64
(transposed) read:
  acc  [c,h,w]: init = x0/8 (ACT), += o=2,4,6 (flip APs, stride ±1)
  accT [c,w,h]: init = x5/8 (ACT; pure transpose == contiguous),
                += o=1,3,7 (flips in transposed coords, stride ±1)
The 1/8 scale folds into every accumulate (DVE STT: term/8 + acc).

Schedule: the kernel is HBM-line-rate bound (72 MiB/core through 16 SDMA
engines at ~26.7 GiB/s each = ~177 us), so the only structural loss is the
serial tail after the LAST load. Each group is split into phase A (o=0,5
inits + o=1,3,7 accT STTs + transposed combine) and phase B (o=2,6,4
accumulated from 16-row chunked loads whose STTs and stores stream with the
DMA). Groups interleave A0,A1,B0,A2,B1,A3,B2,B3 so DVE enters every B phase
caught up; the whole-kernel tail is one chunk STT + one chunk store (~3 us)
instead of the ~16 us combine-last tail.
"""

import sys

for _p in ("/opt/trn_rl_repo",):
    if _p not in sys.path:
        sys.path.insert(0, _p)

import numpy as np

import concourse.bacc as bacc
import concourse.mybir as mybir
from concourse.bass_utils import run_bass_kernel_spmd
from concourse.tile import TileContext

N_CORES = 8
B, C, H, W = 256, 128, 64, 64
ENTRIES_PER_CORE = B // N_CORES          # 32 batch entries
GROUPS_PER_CORE = ENTRIES_PER_CORE // 8  # 4 groups of 8 orientations
RCH = 16                                 # B-phase row-chunk height
ROWS = range(0, H, RCH)


def build_nc(groups: int = GROUPS_PER_CORE) -> bacc.Bacc:
    f32 = mybir.dt.float32
    nc = bacc.Bacc()
    x = nc.declare_dram_parameter("x", [groups * 8, C, H, W], f32, isOutput=False)
    y = nc.declare_dram_parameter("y", [groups, C, H, W], f32, isOutput=True)

    accT_slice = {1: lambda t: t[:, :, ::-1], 3: lambda t: t[:, ::-1, :],
                  7: lambda t: t[:, ::-1, ::-1]}
    # B-phase: for an input row-chunk [r0, r0+RCH) of orientation o, the
    # output rows it feeds and the in-chunk access pattern:
    #   o=2: h0 = 48-r0, chunk[::-1, ::-1]   o=6: h0 = 48-r0, chunk[::-1, :]
    #   o=4: h0 = r0,    chunk[:, ::-1]
    bslice = {2: (lambda r0: H - RCH - r0, lambda t: t[:, ::-1, ::-1]),
              6: (lambda r0: H - RCH - r0, lambda t: t[:, ::-1, :]),
              4: (lambda r0: r0, lambda t: t[:, :, ::-1])}
    mult, add = mybir.AluOpType.mult, mybir.AluOpType.add
    with TileContext(nc) as tc:
        with (
            tc.tile_pool(name="xin", bufs=4) as xin_pool,
            tc.tile_pool(name="bchunk", bufs=12) as bchunk_pool,
            tc.tile_pool(name="acc", bufs=3) as acc_pool,
            tc.tile_pool(name="accT", bufs=2) as accT_pool,
        ):
            accs = {}

            def phase_a(g):
                acc = acc_pool.tile([C, H, W], f32, tag="acc")
                accT = accT_pool.tile([C, H, W], f32, tag="accT")
                accs[g] = acc
                for o in (0, 5, 1, 3, 7):
                    xt = xin_pool.tile([C, H, W], f32, tag="xin")
                    nc.sync.dma_start(xt[:, :, :], x[8 * g + o])
                    if o == 0:
                        nc.scalar.mul(acc[:, :, :], xt[:, :, :], 0.125)
                    elif o == 5:
                        nc.scalar.mul(accT[:, :, :], xt[:, :, :], 0.125)
                    else:
                        nc.vector.scalar_tensor_tensor(
                            accT[:, :, :], accT_slice[o](xt), 0.125,
                            accT[:, :, :], mult, add,
                        )
                # Transposed combine, in H-halves (accT frees after this).
                for h0 in (0, H // 2):
                    hs = slice(h0, h0 + H // 2)
                    nc.vector.tensor_add(
                        acc[:, hs, :], acc[:, hs, :],
                        accT[:, :, hs].transpose([0, 2, 1]),
                    )

            def phase_b(g):
                acc = accs.pop(g)
                for o in (2, 6, 4):
                    h0_of, view = bslice[o]
                    for r0 in ROWS:
                        ct = bchunk_pool.tile([C, RCH, W], f32, tag="bchunk")
                        nc.sync.dma_start(ct[:, :, :], x[8 * g + o][:, r0:r0 + RCH, :])
                        h0 = h0_of(r0)
                        hs = slice(h0, h0 + RCH)
                        nc.vector.scalar_tensor_tensor(
                            acc[:, hs, :], view(ct), 0.125,
                            acc[:, hs, :], mult, add,
                        )
                        if o == 4:
                            # Rows hs are complete once their o=4 chunk lands
                            # (o=2/6 STTs precede in DVE program order).
                            nc.scalar.dma_start(y[g][:, hs, :], acc[:, hs, :])

            phase_a(0)
            phase_a(1)
            phase_b(0)
            phase_a(2)
            phase_b(1)
            phase_a(3)
            phase_b(2)
            phase_b(3)
    nc.compile()
    return nc


_NC_CACHE: list = []


def run(x: np.ndarray, trace: bool = False, **spmd_kwargs):
    """Shard, run on all 8 cores, gather. Returns (output, BassKernelResults)."""
    x = np.ascontiguousarray(x, dtype=np.float32)
    assert x.shape == (B, C, H, W), x.shape
    shards = x.reshape(N_CORES, ENTRIES_PER_CORE, C, H, W)
    if not _NC_CACHE:
        _NC_CACHE.append(build_nc())
    nc = _NC_CACHE[0]
    in_maps = [{"x": shards[i]} for i in range(N_CORES)]
    res = run_bass_kernel_spmd(
        nc, in_maps, list(range(N_CORES)), trace=trace, **spmd_kwargs
    )
    out = np.concatenate([res.results[i]["y"] for i in range(N_CORES)], axis=0)
    return out, res


def kernel(x: np.ndarray) -> np.ndarray:
    out, _ = run(x)
    return out
